# revision 35
# baseline (speedup 1.0000x reference)
"""AdaptiveFrequencyDecomposition Trainium2 kernel (8 NeuronCores, pure data parallel).

Self-contained: hardcodes shapes B,C,H,W = 16,3,512,512, shards batch over 8 cores
(2 batches/core). Per 128x512 image tile the whole DCT->mask->IDCT chain runs as
4 matmul stages; transposes are fused into matmuls by using the *data* as the
stationary (lhsT) operand:
  stage1: psum1[:,128j:] = x_chunk_j^T @ blockdiag(D^T)   (col-DCT + transpose)
  stage2: psum2 = blockdiag(D^T)^T @ sb1                  (row-DCT) -> coeffs C^T layout
  stage3: psum3[:,128j:] = (mask*C^T)_chunk_j^T @ blockdiag(D)  (row-IDCT + transpose)
  stage4: psum4 = blockdiag(D)^T @ sb3                    (col-IDCT) -> natural layout
mid band is exact by linearity: mid = x - low - high (the reference's clip never
activates for these threshold ranges: max(low_mask+high_mask) < 1).
"""
import math
import os
import sys

for _p in ("/opt/trn_rl_repo",):
    if _p not in sys.path and os.path.isdir(_p):
        sys.path.append(_p)

import numpy as np

B, C, H, W = 16, 3, 512, 512
NCORES = 8
BPC = B // NCORES  # batches per core = 2


def _make_dct(n=8):
    d = np.zeros((n, n), dtype=np.float32)
    for k in range(n):
        for i in range(n):
            if k == 0:
                d[k, i] = 1.0 / math.sqrt(n)
            else:
                d[k, i] = math.sqrt(2.0 / n) * math.cos(math.pi * k * (2 * i + 1) / (2 * n))
    return d


def _make_zigzag(n=8):
    z = np.zeros((n, n), dtype=np.float32)
    i, j = 0, 0
    for idx in range(n * n):
        z[i, j] = idx
        if (i + j) % 2 == 0:
            if j == n - 1:
                i += 1
            elif i == 0:
                j += 1
            else:
                i -= 1
                j += 1
        elif i == n - 1:
            j += 1
        elif j == 0:
            i += 1
        else:
            i += 1
            j -= 1
    return z / (n * n - 1)


def _consts():
    D = _make_dct()
    ZZ = _make_zigzag()
    BDT = np.zeros((128, 128), np.float32)
    BDD = np.zeros((128, 128), np.float32)
    for g in range(16):
        BDT[g * 8:(g + 1) * 8, g * 8:(g + 1) * 8] = D.T
        BDD[g * 8:(g + 1) * 8, g * 8:(g + 1) * 8] = D
    p_idx = np.arange(128)
    f_idx = np.arange(512)
    zzT = ZZ[np.ix_(f_idx % 8, p_idx % 8)].T.astype(np.float32).copy()  # [128,512]
    ONESP = np.zeros((128, 32), np.float32)  # 4 blocks of [128, 8]
    for t in range(4):
        ONESP[:64, t * 8 + 2 * t] = 1.0
        ONESP[64:, t * 8 + 2 * t + 1] = 1.0
    ID8 = np.eye(8, dtype=np.float32)
    ID2 = np.eye(2, dtype=np.float32)
    ONES1 = np.ones((1, 128), np.float32)
    STHR = np.array([[12.5], [-12.5]], np.float32)
    BTHR = np.array([[7.5], [-30.0]], np.float32)
    SOUT = np.array([[0.25], [0.25]], np.float32)
    BOUT = np.array([[0.15], [0.6]], np.float32)
    return dict(BDT=BDT, BDD=BDD, zzT=zzT, ONESP=ONESP, ID8=ID8, ID2=ID2,
                ONES1=ONES1, STHR=STHR, BTHR=BTHR, SOUT=SOUT, BOUT=BOUT)


_CACHE = {}


def _build():
    """Build + compile the Bacc graph (once)."""
    if "nc" in _CACHE:
        return _CACHE["nc"]
    import concourse.bass as bass
    import concourse.mybir as mybir
    import concourse.tile as tile
    from concourse import bacc

    f32 = mybir.dt.float32
    f16 = mybir.dt.float16
    nc = bacc.Bacc("TRN2", target_bir_lowering=False, debug=False, num_devices=NCORES)

    # --- DRAM parameters ---
    x_d = nc.dram_tensor("x", [BPC, C, H, W], f16, kind="ExternalInput").ap()
    c16_d = nc.dram_tensor("C16", [128, 288], f16, kind="ExternalInput").ap()
    c32_d = nc.dram_tensor("C32", [128, 1088], f32, kind="ExternalInput").ap()

    low_d = nc.dram_tensor("low", [BPC, C, H, W], f32, kind="ExternalOutput").ap()
    mid_d = nc.dram_tensor("mid", [BPC, C, H, W], f32, kind="ExternalOutput").ap()
    high_d = nc.dram_tensor("high", [BPC, C, H, W], f32, kind="ExternalOutput").ap()
    lowt_d = nc.dram_tensor("low_t", [BPC, 1], f32, kind="ExternalOutput").ap()
    hight_d = nc.dram_tensor("high_t", [BPC, 1], f32, kind="ExternalOutput").ap()
    warm_d = nc.dram_tensor("warm", [128, 1], f32, kind="ExternalOutput").ap()


    Sig = mybir.ActivationFunctionType.Sigmoid
    Relu = mybir.ActivationFunctionType.Relu
    Ident = mybir.ActivationFunctionType.Identity
    AX = mybir.AxisListType.X

    with tile.TileContext(nc) as tc:
        with (
            tc.tile_pool(name="consts", bufs=1) as consts,
            tc.tile_pool(name="xin", bufs=7) as xin,
            tc.tile_pool(name="stage", bufs=3) as stagep,
            tc.tile_pool(name="coeff", bufs=12) as coeffp,
            tc.tile_pool(name="tmp", bufs=6) as tmpp,
            tc.tile_pool(name="tmpb", bufs=6) as tmpbp,
            tc.tile_pool(name="small", bufs=2) as smallp,
            tc.tile_pool(name="masks", bufs=2) as maskp,
            tc.tile_pool(name="ps1", bufs=1, space="PSUM") as ps1p,
            tc.tile_pool(name="ps2", bufs=2, space="PSUM") as ps2p,
            tc.tile_pool(name="ps3", bufs=2, space="PSUM") as ps3p,
            tc.tile_pool(name="ps4", bufs=2, space="PSUM") as ps4p,
            tc.tile_pool(name="psm", bufs=1, space="PSUM") as psmp,
        ):
            # --- PE warmup: dense junk matmul burst so HAM is at 2.4 GHz
            # when the real chains start (sunk to a dummy output vs DCE) ---
            warm_src = smallp.tile([128, 128], f32, tag="wsrc", name="warm_src")
            nc.vector.memset(warm_src, 1.0)
            warm_ps = ps4p.tile([128, 128], f32, tag="ps4", name="warm_ps")
            for i in range(10):
                nc.tensor.matmul(warm_ps, warm_src, warm_src,
                                 start=(i == 0), stop=(i == 9))
            warm_sb = smallp.tile([128, 1], f32, tag="wout", name="warm_sb")
            nc.vector.tensor_copy(out=warm_sb, in_=warm_ps[:, 0:1])
            nc.scalar.dma_start(out=warm_d, in_=warm_sb)

            # --- constants: two packed blob DMAs, sliced into views ---
            c16_sb = consts.tile([128, 288], f16, tag="c16", name="c16")
            nc.sync.dma_start(out=c16_sb, in_=c16_d)
            c32_sb = consts.tile([128, 1088], f32, tag="c32", name="c32")
            nc.sync.dma_start(out=c32_sb, in_=c32_d)
            bdt_sb = c16_sb[:, 0:128]
            bdd_sb = c16_sb[:, 128:256]
            onesp_sb = c16_sb[:, 256:288]
            zzt_sb = c32_sb[:, 0:512]
            w1p_sb = c32_sb[0:24, 512:768]
            id8_sb = c32_sb[0:8, 768:776]
            id2_sb = c32_sb[0:2, 784:786]
            ones1_sb = c32_sb[0:1, 800:928]
            w2_sb = c32_sb[0:32, 928:930]
            b1_sb = c32_sb[0:32, 944:945]
            b2_sb = c32_sb[0:2, 960:961]
            sthr_sb = c32_sb[0:2, 976:977]
            bthr_sb = c32_sb[0:2, 992:993]
            sout_sb = c32_sb[0:2, 1008:1009]
            bout_sb = c32_sb[0:2, 1024:1025]

            state = {}  # per-batch: x tiles, masks

            def load_imgs(b):
                st = state.setdefault(b, {"x": {}})
                for c in range(C):
                    x_sb = xin.tile([128, 4, 512], f16, tag="x", name=f"x_{b}_{c}")
                    nc.scalar.dma_start(
                        out=x_sb,
                        in_=x_d[b, c].rearrange("(t p) w -> p t w", p=128),
                    )
                    st["x"][c] = x_sb

            def pool_img(b, c):
                st = state[b]
                x_sb = st["x"][c]
                pool_ps = psmp.tile([8, 512], f32, tag="psm", name=f"poolps_{b}_{c}")
                for t in range(4):
                    nc.tensor.matmul(
                        pool_ps,
                        onesp_sb[:, t * 8:(t + 1) * 8],
                        x_sb[:, t, :],
                        start=(t == 0),
                        stop=(t == 3),
                    )
                pooled_all = st.setdefault(
                    "pooled",
                    smallp.tile([8, 24], f32, tag="pooled_all", name=f"pooled_{b}"))
                nc.vector.reduce_sum(
                    out=pooled_all[:, c * 8:(c + 1) * 8],
                    in_=pool_ps.rearrange("p (jj w) -> p jj w", w=64),
                    axis=AX,
                )

            def mlp(b):
                st = state[b]
                pooled_all = st["pooled"]
                pooledT_ps = psmp.tile([24, 8], f32, tag="psm", name=f"pooledT_ps_{b}")
                nc.tensor.matmul(pooledT_ps, pooled_all, id8_sb)
                pooledT_sb = smallp.tile([24, 8], f32, tag="pooledT", name=f"pooledT_{b}")
                nc.vector.tensor_copy(out=pooledT_sb, in_=pooledT_ps)
                # h[m] = sum_i sum_(c,jj) pooledT[(c jj), i] * W1P[(c jj), i, m]
                h_ps = psmp.tile([32, 1], f32, tag="psm", name=f"h_ps_{b}")
                for i in range(8):
                    nc.tensor.matmul(
                        h_ps, w1p_sb[:, i * 32:(i + 1) * 32], pooledT_sb[:, i:i + 1],
                        start=(i == 0), stop=(i == 7),
                    )
                h_sb = smallp.tile([32, 1], f32, tag="h", name=f"h_{b}")
                nc.scalar.activation(out=h_sb, in_=h_ps, func=Relu, bias=b1_sb, scale=1.0)
                t_ps = psmp.tile([2, 1], f32, tag="psm", name=f"t_ps_{b}")
                nc.tensor.matmul(t_ps, w2_sb, h_sb)
                tt_sb = smallp.tile([2, 1], f32, tag="tt", name=f"tt_{b}")
                nc.scalar.activation(out=tt_sb, in_=t_ps, func=Sig, bias=b2_sb, scale=1.0)
                # thr = tt*[12.5,-12.5] + [7.5,-30]  (= [50*low_t, -50*high_t])
                thr_sb = smallp.tile([2, 1], f32, tag="thr", name=f"thr_{b}")
                nc.scalar.activation(out=thr_sb, in_=tt_sb, func=Ident,
                                     bias=bthr_sb, scale=sthr_sb)
                # unscaled thresholds out
                tout_sb = smallp.tile([2, 1], f32, tag="tout", name=f"tout_{b}")
                nc.scalar.activation(out=tout_sb, in_=tt_sb, func=Ident,
                                     bias=bout_sb, scale=sout_sb)
                nc.sync.dma_start(out=lowt_d[b:b + 1, :], in_=tout_sb[0:1, 0:1])
                nc.sync.dma_start(out=hight_d[b:b + 1, :], in_=tout_sb[1:2, 0:1])
                # broadcast thresholds across partitions: transpose then ones-matmul
                thrT_ps = psmp.tile([1, 2], f32, tag="psm", name=f"thrT_ps_{b}")
                nc.tensor.matmul(thrT_ps, thr_sb, id2_sb)
                thrT_sb = smallp.tile([1, 2], f32, tag="thrT", name=f"thrT_{b}")
                nc.vector.tensor_copy(out=thrT_sb, in_=thrT_ps)
                bc_ps = psmp.tile([128, 2], f32, tag="psm", name=f"bc_ps_{b}")
                nc.tensor.matmul(bc_ps, ones1_sb, thrT_sb)
                bc_sb = smallp.tile([128, 2], f32, tag="bc", name=f"bc_{b}")
                nc.vector.tensor_copy(out=bc_sb, in_=bc_ps)
                mask_lo = maskp.tile([128, 512], f16, tag="mlo", name=f"mlo_{b}")
                nc.scalar.activation(out=mask_lo, in_=zzt_sb, func=Sig,
                                     bias=bc_sb[:, 0:1], scale=-50.0)
                mask_hi = maskp.tile([128, 512], f16, tag="mhi", name=f"mhi_{b}")
                nc.scalar.activation(out=mask_hi, in_=zzt_sb, func=Sig,
                                     bias=bc_sb[:, 1:2], scale=50.0)
                st["mask_lo"] = mask_lo
                st["mask_hi"] = mask_hi

            def dct_tile(b, c, t, x_sb):
                """stage1+stage2 for one tile; returns the coeff psum tile."""
                ps1 = ps1p.tile([128, 512], f32, tag="ps1", name=f"ps1_{b}_{c}_{t}")
                for j in range(4):
                    nc.tensor.matmul(
                        ps1[:, 128 * j:128 * (j + 1)],
                        x_sb[:, t, 128 * j:128 * (j + 1)],
                        bdt_sb,
                    )
                sb1 = tmpp.tile([128, 512], f16, tag="sb1", name=f"sb1_{b}_{c}_{t}")
                nc.any.tensor_copy(out=sb1, in_=ps1)
                ps2 = ps2p.tile([128, 512], f32, tag="ps2", name=f"ps2_{b}_{c}_{t}")
                nc.tensor.matmul(ps2, bdt_sb, sb1)
                return ps2

            def dct_img(b, c):
                """early DCT: stage coeffs to SBUF (before masks are known)."""
                st = state[b]
                cf = st.setdefault("coeff", {})
                for t in range(4):
                    ps2 = dct_tile(b, c, t, st["x"][c])
                    coeff_sb = coeffp.tile([128, 512], f16, tag="coeff",
                                           name=f"coeff_{b}_{c}_{t}")
                    nc.any.tensor_copy(out=coeff_sb, in_=ps2)
                    cf[(c, t)] = coeff_sb

            def inv_half(b, c, t, msk_sl, st_sb, mn):
                ps3 = ps3p.tile([128, 512], f32, tag="ps3",
                                name=f"ps3_{b}_{c}_{t}_{mn}")
                for j in range(4):
                    nc.tensor.matmul(
                        ps3[:, 128 * j:128 * (j + 1)],
                        msk_sl[:, 128 * j:128 * (j + 1)],
                        bdd_sb,
                    )
                sb3 = tmpp.tile([128, 512], f16, tag="sb3",
                                name=f"sb3_{b}_{c}_{t}_{mn}")
                nc.any.tensor_copy(out=sb3, in_=ps3)
                ps4 = ps4p.tile([128, 512], f32, tag="ps4",
                                name=f"ps4_{b}_{c}_{t}_{mn}")
                nc.tensor.matmul(ps4, bdd_sb, sb3)
                nc.any.tensor_copy(out=st_sb[:, t, :], in_=ps4)

            def finish_tile(b, c, t, coeff_src, st_lo, st_hi, st_mid, x_sb,
                            mid_eng=None):
                for mask_key, st_sb, mn in (("mask_lo", st_lo, "lo"),
                                            ("mask_hi", st_hi, "hi")):
                    msk = tmpbp.tile([128, 512], f16, tag="msk",
                                     name=f"msk_{b}_{c}_{t}_{mn}")
                    nc.vector.tensor_mul(msk, coeff_src, state[b][mask_key])
                    inv_half(b, c, t, msk, st_sb, mn)
                if mid_eng is None:
                    tmid = tmpbp.tile([128, 512], f16, tag="tmid",
                                      name=f"tmid_{b}_{c}_{t}")
                    nc.gpsimd.tensor_sub(tmid, x_sb[:, t, :], st_lo[:, t, :])
                    nc.gpsimd.tensor_sub(st_mid[:, t, :], tmid, st_hi[:, t, :])
                else:
                    tmid = tmpbp.tile([128, 512], f32, tag="tmidv",
                                      name=f"tmidv_{b}_{c}_{t}")
                    mid_eng.tensor_sub(tmid, x_sb[:, t, :], st_lo[:, t, :])
                    mid_eng.tensor_sub(st_mid[:, t, :], tmid, st_hi[:, t, :])

            def out_dmas(b, c, st_lo, st_hi, st_mid, split_mid=False):
                for st_sb, out_d in ((st_lo, low_d), (st_hi, high_d)):
                    oo = out_d[b, c].rearrange("(t p) w -> p t w", p=128)
                    nc.sync.dma_start(out=oo[:, 0:2, :], in_=st_sb[:, 0:2, :])
                    nc.sync.dma_start(out=oo[:, 2:4, :], in_=st_sb[:, 2:4, :])
                if split_mid:
                    mo = mid_d[b, c].rearrange("(t p) w -> p t w", p=128)
                    for t in range(4):
                        nc.sync.dma_start(out=mo[:, t:t + 1, :],
                                          in_=st_mid[:, t:t + 1, :])
                else:
                    mo = mid_d[b, c].rearrange("(t p) w -> p t w", p=128)
                    nc.sync.dma_start(out=mo[:, 0:2, :], in_=st_mid[:, 0:2, :])
                    nc.sync.dma_start(out=mo[:, 2:4, :], in_=st_mid[:, 2:4, :])

            def stage_tiles(b, c):
                st_lo = stagep.tile([128, 4, 512], f32, tag="stlo", name=f"stlo_{b}_{c}")
                st_hi = stagep.tile([128, 4, 512], f32, tag="sthi", name=f"sthi_{b}_{c}")
                st_mid = stagep.tile([128, 4, 512], f32, tag="stmid", name=f"stmid_{b}_{c}")
                return st_lo, st_hi, st_mid

            def finish_img(b, c):
                """apply masks + inverse transforms to staged coeffs."""
                st = state[b]
                st_lo, st_hi, st_mid = stage_tiles(b, c)
                for t in range(4):
                    finish_tile(b, c, t, st["coeff"][(c, t)],
                                st_lo, st_hi, st_mid, st["x"][c])
                out_dmas(b, c, st_lo, st_hi, st_mid)

            def chain_img(b, c, last=False):
                """full fused chain: DCT -> mask(from psum) -> inverse."""
                st = state[b]
                st_lo, st_hi, st_mid = stage_tiles(b, c)
                for t in range(4):
                    ps2 = dct_tile(b, c, t, st["x"][c])
                    finish_tile(b, c, t, ps2, st_lo, st_hi, st_mid, st["x"][c],
                                mid_eng=nc.vector if last else None)
                out_dmas(b, c, st_lo, st_hi, st_mid, split_mid=last)

            # software-pipelined program order across the 2 batches
            load_imgs(0)
            load_imgs(1)
            dct_img(0, 0)
            pool_img(0, 0)
            dct_img(0, 1)
            pool_img(0, 1)
            pool_img(0, 2)
            mlp(0)
            finish_img(0, 0)
            pool_img(1, 0)
            finish_img(0, 1)
            pool_img(1, 1)
            chain_img(0, 2)
            pool_img(1, 2)
            mlp(1)
            chain_img(1, 0)
            chain_img(1, 1)
            chain_img(1, 2, last=True)

    nc.compile()
    _CACHE["nc"] = nc
    return nc


def kernel(**inputs):
    from concourse.bass_utils import run_bass_kernel_spmd

    x = np.ascontiguousarray(np.asarray(inputs["x"], dtype=np.float32).astype(np.float16))
    w1 = np.asarray(inputs["w1"], dtype=np.float32)
    b1 = np.asarray(inputs["b1"], dtype=np.float32)
    w2 = np.asarray(inputs["w2"], dtype=np.float32)
    b2 = np.asarray(inputs["b2"], dtype=np.float32)

    cst = _consts()
    nc = _build()

    # W1P[(c*8+jj), i, m] = w1[c*64+i*8+jj, m] / 4096
    w1s = (w1 / 4096.0).astype(np.float32).reshape(3, 8, 8, 32)  # [c, i, jj, m]
    W1P = np.ascontiguousarray(
        w1s.transpose(0, 2, 1, 3).reshape(24, 256))  # [(c jj), (i m)]
    C16 = np.zeros((128, 288), np.float16)
    C16[:, 0:128] = cst["BDT"].astype(np.float16)
    C16[:, 128:256] = cst["BDD"].astype(np.float16)
    C16[:, 256:288] = cst["ONESP"].astype(np.float16)
    C32 = np.zeros((128, 1088), np.float32)
    C32[:, 0:512] = cst["zzT"]
    C32[0:24, 512:768] = W1P
    C32[0:8, 768:776] = cst["ID8"]
    C32[0:2, 784:786] = cst["ID2"]
    C32[0:1, 800:928] = cst["ONES1"]
    C32[0:32, 928:930] = w2
    C32[0:32, 944:945] = b1.reshape(32, 1)
    C32[0:2, 960:961] = b2.reshape(2, 1)
    C32[0:2, 976:977] = cst["STHR"]
    C32[0:2, 992:993] = cst["BTHR"]
    C32[0:2, 1008:1009] = cst["SOUT"]
    C32[0:2, 1024:1025] = cst["BOUT"]
    base = {"C16": C16, "C32": C32}
    in_maps = []
    for i in range(NCORES):
        m = dict(base)
        m["x"] = np.ascontiguousarray(x[i * BPC:(i + 1) * BPC])
        in_maps.append(m)

    res = run_bass_kernel_spmd(nc, in_maps, core_ids=list(range(NCORES)))
    low = np.concatenate([res.results[i]["low"] for i in range(NCORES)], axis=0)
    mid = np.concatenate([res.results[i]["mid"] for i in range(NCORES)], axis=0)
    high = np.concatenate([res.results[i]["high"] for i in range(NCORES)], axis=0)
    low_t = np.concatenate([res.results[i]["low_t"] for i in range(NCORES)], axis=0)
    high_t = np.concatenate([res.results[i]["high_t"] for i in range(NCORES)], axis=0)
    return low, mid, high, (low_t, high_t)


# revision 36
# speedup vs baseline: 1.0320x; 1.0320x over previous
"""AdaptiveFrequencyDecomposition Trainium2 kernel (8 NeuronCores, pure data parallel).

Self-contained: hardcodes shapes B,C,H,W = 16,3,512,512, shards batch over 8 cores
(2 batches/core). Per 128x512 image tile the whole DCT->mask->IDCT chain runs as
4 matmul stages; transposes are fused into matmuls by using the *data* as the
stationary (lhsT) operand:
  stage1: psum1[:,128j:] = x_chunk_j^T @ blockdiag(D^T)   (col-DCT + transpose)
  stage2: psum2 = blockdiag(D^T)^T @ sb1                  (row-DCT) -> coeffs C^T layout
  stage3: psum3[:,128j:] = (mask*C^T)_chunk_j^T @ blockdiag(D)  (row-IDCT + transpose)
  stage4: psum4 = blockdiag(D)^T @ sb3                    (col-IDCT) -> natural layout
mid band is exact by linearity: mid = x - low - high (the reference's clip never
activates for these threshold ranges: max(low_mask+high_mask) < 1).
"""
import math
import os
import sys

for _p in ("/opt/trn_rl_repo",):
    if _p not in sys.path and os.path.isdir(_p):
        sys.path.append(_p)

import numpy as np

B, C, H, W = 16, 3, 512, 512
NCORES = 8
BPC = B // NCORES  # batches per core = 2


def _make_dct(n=8):
    d = np.zeros((n, n), dtype=np.float32)
    for k in range(n):
        for i in range(n):
            if k == 0:
                d[k, i] = 1.0 / math.sqrt(n)
            else:
                d[k, i] = math.sqrt(2.0 / n) * math.cos(math.pi * k * (2 * i + 1) / (2 * n))
    return d


def _make_zigzag(n=8):
    z = np.zeros((n, n), dtype=np.float32)
    i, j = 0, 0
    for idx in range(n * n):
        z[i, j] = idx
        if (i + j) % 2 == 0:
            if j == n - 1:
                i += 1
            elif i == 0:
                j += 1
            else:
                i -= 1
                j += 1
        elif i == n - 1:
            j += 1
        elif j == 0:
            i += 1
        else:
            i += 1
            j -= 1
    return z / (n * n - 1)


def _consts():
    D = _make_dct()
    ZZ = _make_zigzag()
    BDT = np.zeros((128, 128), np.float32)
    BDD = np.zeros((128, 128), np.float32)
    for g in range(16):
        BDT[g * 8:(g + 1) * 8, g * 8:(g + 1) * 8] = D.T
        BDD[g * 8:(g + 1) * 8, g * 8:(g + 1) * 8] = D
    p_idx = np.arange(128)
    f_idx = np.arange(512)
    zzT = ZZ[np.ix_(f_idx % 8, p_idx % 8)].T.astype(np.float32).copy()  # [128,512]
    ONESP = np.zeros((128, 32), np.float32)  # 4 blocks of [128, 8]
    for t in range(4):
        ONESP[:64, t * 8 + 2 * t] = 1.0
        ONESP[64:, t * 8 + 2 * t + 1] = 1.0
    ID8 = np.eye(8, dtype=np.float32)
    ID2 = np.eye(2, dtype=np.float32)
    ONES1 = np.ones((1, 128), np.float32)
    STHR = np.array([[12.5], [-12.5]], np.float32)
    BTHR = np.array([[7.5], [-30.0]], np.float32)
    SOUT = np.array([[0.25], [0.25]], np.float32)
    BOUT = np.array([[0.15], [0.6]], np.float32)
    return dict(BDT=BDT, BDD=BDD, zzT=zzT, ONESP=ONESP, ID8=ID8, ID2=ID2,
                ONES1=ONES1, STHR=STHR, BTHR=BTHR, SOUT=SOUT, BOUT=BOUT)


_CACHE = {}


def _build():
    """Build + compile the Bacc graph (once)."""
    if "nc" in _CACHE:
        return _CACHE["nc"]
    import concourse.bass as bass
    import concourse.mybir as mybir
    import concourse.tile as tile
    from concourse import bacc

    f32 = mybir.dt.float32
    f16 = mybir.dt.float16
    nc = bacc.Bacc("TRN2", target_bir_lowering=False, debug=False, num_devices=NCORES)

    # --- DRAM parameters ---
    x_d = nc.dram_tensor("x", [BPC, C, H, W], f16, kind="ExternalInput").ap()
    c16_d = nc.dram_tensor("C16", [128, 288], f16, kind="ExternalInput").ap()
    c32_d = nc.dram_tensor("C32", [128, 1088], f32, kind="ExternalInput").ap()

    low_d = nc.dram_tensor("low", [BPC, C, H, W], f32, kind="ExternalOutput").ap()
    mid_d = nc.dram_tensor("mid", [BPC, C, H, W], f32, kind="ExternalOutput").ap()
    high_d = nc.dram_tensor("high", [BPC, C, H, W], f32, kind="ExternalOutput").ap()
    lowt_d = nc.dram_tensor("low_t", [BPC, 1], f32, kind="ExternalOutput").ap()
    hight_d = nc.dram_tensor("high_t", [BPC, 1], f32, kind="ExternalOutput").ap()
    warm_d = nc.dram_tensor("warm", [128, 1], f32, kind="ExternalOutput").ap()


    Sig = mybir.ActivationFunctionType.Sigmoid
    Relu = mybir.ActivationFunctionType.Relu
    Ident = mybir.ActivationFunctionType.Identity
    AX = mybir.AxisListType.X

    with tile.TileContext(nc) as tc:
        with (
            tc.tile_pool(name="consts", bufs=1) as consts,
            tc.tile_pool(name="xin", bufs=7) as xin,
            tc.tile_pool(name="stage", bufs=3) as stagep,
            tc.tile_pool(name="coeff", bufs=12) as coeffp,
            tc.tile_pool(name="tmp", bufs=6) as tmpp,
            tc.tile_pool(name="tmpb", bufs=6) as tmpbp,
            tc.tile_pool(name="small", bufs=2) as smallp,
            tc.tile_pool(name="masks", bufs=2) as maskp,
            tc.tile_pool(name="ps1", bufs=1, space="PSUM") as ps1p,
            tc.tile_pool(name="ps2", bufs=2, space="PSUM") as ps2p,
            tc.tile_pool(name="ps3", bufs=2, space="PSUM") as ps3p,
            tc.tile_pool(name="ps4", bufs=2, space="PSUM") as ps4p,
            tc.tile_pool(name="psm", bufs=1, space="PSUM") as psmp,
        ):
            # --- PE warmup: dense junk matmul burst so HAM is at 2.4 GHz
            # when the real chains start (sunk to a dummy output vs DCE) ---
            warm_src = smallp.tile([128, 128], f32, tag="wsrc", name="warm_src")
            nc.vector.memset(warm_src, 1.0)
            warm_ps = ps4p.tile([128, 128], f32, tag="ps4", name="warm_ps")
            for i in range(10):
                nc.tensor.matmul(warm_ps, warm_src, warm_src,
                                 start=(i == 0), stop=(i == 9))
            warm_sb = smallp.tile([128, 1], f32, tag="wout", name="warm_sb")
            nc.vector.tensor_copy(out=warm_sb, in_=warm_ps[:, 0:1])
            nc.scalar.dma_start(out=warm_d, in_=warm_sb)

            # --- constants: two packed blob DMAs, sliced into views ---
            c16_sb = consts.tile([128, 288], f16, tag="c16", name="c16")
            nc.sync.dma_start(out=c16_sb, in_=c16_d)
            c32_sb = consts.tile([128, 1088], f32, tag="c32", name="c32")
            nc.sync.dma_start(out=c32_sb, in_=c32_d)
            bdt_sb = c16_sb[:, 0:128]
            bdd_sb = c16_sb[:, 128:256]
            onesp_sb = c16_sb[:, 256:288]
            zzt_sb = c32_sb[:, 0:512]
            w1p_sb = c32_sb[0:24, 512:768]
            id8_sb = c32_sb[0:8, 768:776]
            id2_sb = c32_sb[0:2, 784:786]
            ones1_sb = c32_sb[0:1, 800:928]
            w2_sb = c32_sb[0:32, 928:930]
            b1_sb = c32_sb[0:32, 944:945]
            b2_sb = c32_sb[0:2, 960:961]
            sthr_sb = c32_sb[0:2, 976:977]
            bthr_sb = c32_sb[0:2, 992:993]
            sout_sb = c32_sb[0:2, 1008:1009]
            bout_sb = c32_sb[0:2, 1024:1025]

            state = {}  # per-batch: x tiles, masks

            def load_imgs(b):
                st = state.setdefault(b, {"x": {}})
                for c in range(C):
                    x_sb = xin.tile([128, 4, 512], f16, tag="x", name=f"x_{b}_{c}")
                    nc.scalar.dma_start(
                        out=x_sb,
                        in_=x_d[b, c].rearrange("(t p) w -> p t w", p=128),
                    )
                    st["x"][c] = x_sb

            def pool_img(b, c):
                st = state[b]
                x_sb = st["x"][c]
                pool_ps = psmp.tile([8, 512], f32, tag="psm", name=f"poolps_{b}_{c}")
                for t in range(4):
                    nc.tensor.matmul(
                        pool_ps,
                        onesp_sb[:, t * 8:(t + 1) * 8],
                        x_sb[:, t, :],
                        start=(t == 0),
                        stop=(t == 3),
                    )
                pooled_all = st.setdefault(
                    "pooled",
                    smallp.tile([8, 24], f32, tag="pooled_all", name=f"pooled_{b}"))
                nc.vector.reduce_sum(
                    out=pooled_all[:, c * 8:(c + 1) * 8],
                    in_=pool_ps.rearrange("p (jj w) -> p jj w", w=64),
                    axis=AX,
                )

            def mlp(b):
                st = state[b]
                pooled_all = st["pooled"]
                pooledT_ps = psmp.tile([24, 8], f32, tag="psm", name=f"pooledT_ps_{b}")
                nc.tensor.matmul(pooledT_ps, pooled_all, id8_sb)
                pooledT_sb = smallp.tile([24, 8], f32, tag="pooledT", name=f"pooledT_{b}")
                nc.vector.tensor_copy(out=pooledT_sb, in_=pooledT_ps)
                # h[m] = sum_i sum_(c,jj) pooledT[(c jj), i] * W1P[(c jj), i, m]
                h_ps = psmp.tile([32, 1], f32, tag="psm", name=f"h_ps_{b}")
                for i in range(8):
                    nc.tensor.matmul(
                        h_ps, w1p_sb[:, i * 32:(i + 1) * 32], pooledT_sb[:, i:i + 1],
                        start=(i == 0), stop=(i == 7),
                    )
                h_sb = smallp.tile([32, 1], f32, tag="h", name=f"h_{b}")
                nc.scalar.activation(out=h_sb, in_=h_ps, func=Relu, bias=b1_sb, scale=1.0)
                t_ps = psmp.tile([2, 1], f32, tag="psm", name=f"t_ps_{b}")
                nc.tensor.matmul(t_ps, w2_sb, h_sb)
                tt_sb = smallp.tile([2, 1], f32, tag="tt", name=f"tt_{b}")
                nc.scalar.activation(out=tt_sb, in_=t_ps, func=Sig, bias=b2_sb, scale=1.0)
                # thr = tt*[12.5,-12.5] + [7.5,-30]  (= [50*low_t, -50*high_t])
                thr_sb = smallp.tile([2, 1], f32, tag="thr", name=f"thr_{b}")
                nc.scalar.activation(out=thr_sb, in_=tt_sb, func=Ident,
                                     bias=bthr_sb, scale=sthr_sb)
                # unscaled thresholds out
                tout_sb = smallp.tile([2, 1], f32, tag="tout", name=f"tout_{b}")
                nc.scalar.activation(out=tout_sb, in_=tt_sb, func=Ident,
                                     bias=bout_sb, scale=sout_sb)
                nc.sync.dma_start(out=lowt_d[b:b + 1, :], in_=tout_sb[0:1, 0:1])
                nc.sync.dma_start(out=hight_d[b:b + 1, :], in_=tout_sb[1:2, 0:1])
                # broadcast thresholds across partitions: transpose then ones-matmul
                thrT_ps = psmp.tile([1, 2], f32, tag="psm", name=f"thrT_ps_{b}")
                nc.tensor.matmul(thrT_ps, thr_sb, id2_sb)
                thrT_sb = smallp.tile([1, 2], f32, tag="thrT", name=f"thrT_{b}")
                nc.vector.tensor_copy(out=thrT_sb, in_=thrT_ps)
                bc_ps = psmp.tile([128, 2], f32, tag="psm", name=f"bc_ps_{b}")
                nc.tensor.matmul(bc_ps, ones1_sb, thrT_sb)
                bc_sb = smallp.tile([128, 2], f32, tag="bc", name=f"bc_{b}")
                nc.vector.tensor_copy(out=bc_sb, in_=bc_ps)
                mask_lo = maskp.tile([128, 512], f16, tag="mlo", name=f"mlo_{b}")
                nc.scalar.activation(out=mask_lo, in_=zzt_sb, func=Sig,
                                     bias=bc_sb[:, 0:1], scale=-50.0)
                mask_hi = maskp.tile([128, 512], f16, tag="mhi", name=f"mhi_{b}")
                nc.scalar.activation(out=mask_hi, in_=zzt_sb, func=Sig,
                                     bias=bc_sb[:, 1:2], scale=50.0)
                st["mask_lo"] = mask_lo
                st["mask_hi"] = mask_hi

            def dct_tile(b, c, t, x_sb):
                """stage1+stage2 for one tile; returns the coeff psum tile."""
                ps1 = ps1p.tile([128, 512], f32, tag="ps1", name=f"ps1_{b}_{c}_{t}")
                for j in range(4):
                    nc.tensor.matmul(
                        ps1[:, 128 * j:128 * (j + 1)],
                        x_sb[:, t, 128 * j:128 * (j + 1)],
                        bdt_sb,
                    )
                sb1 = tmpp.tile([128, 512], f16, tag="sb1", name=f"sb1_{b}_{c}_{t}")
                nc.any.tensor_copy(out=sb1, in_=ps1)
                ps2 = ps2p.tile([128, 512], f32, tag="ps2", name=f"ps2_{b}_{c}_{t}")
                nc.tensor.matmul(ps2, bdt_sb, sb1)
                return ps2

            def dct_img(b, c):
                """early DCT: stage coeffs to SBUF (before masks are known)."""
                st = state[b]
                cf = st.setdefault("coeff", {})
                for t in range(4):
                    ps2 = dct_tile(b, c, t, st["x"][c])
                    coeff_sb = coeffp.tile([128, 512], f16, tag="coeff",
                                           name=f"coeff_{b}_{c}_{t}")
                    nc.any.tensor_copy(out=coeff_sb, in_=ps2)
                    cf[(c, t)] = coeff_sb

            def inv_half(b, c, t, msk_sl, st_sb, mn):
                ps3 = ps3p.tile([128, 512], f32, tag="ps3",
                                name=f"ps3_{b}_{c}_{t}_{mn}")
                for j in range(4):
                    nc.tensor.matmul(
                        ps3[:, 128 * j:128 * (j + 1)],
                        msk_sl[:, 128 * j:128 * (j + 1)],
                        bdd_sb,
                    )
                sb3 = tmpp.tile([128, 512], f16, tag="sb3",
                                name=f"sb3_{b}_{c}_{t}_{mn}")
                nc.any.tensor_copy(out=sb3, in_=ps3)
                ps4 = ps4p.tile([128, 512], f32, tag="ps4",
                                name=f"ps4_{b}_{c}_{t}_{mn}")
                nc.tensor.matmul(ps4, bdd_sb, sb3)
                nc.any.tensor_copy(out=st_sb[:, t, :], in_=ps4)

            def finish_tile(b, c, t, coeff_src, st_lo, st_hi, st_mid, x_sb,
                            mid_eng=None):
                for mask_key, st_sb, mn in (("mask_lo", st_lo, "lo"),
                                            ("mask_hi", st_hi, "hi")):
                    msk = tmpbp.tile([128, 512], f16, tag="msk",
                                     name=f"msk_{b}_{c}_{t}_{mn}")
                    nc.vector.tensor_mul(msk, coeff_src, state[b][mask_key])
                    inv_half(b, c, t, msk, st_sb, mn)
                if mid_eng is None:
                    tmid = tmpbp.tile([128, 512], f16, tag="tmid",
                                      name=f"tmid_{b}_{c}_{t}")
                    nc.gpsimd.tensor_sub(tmid, x_sb[:, t, :], st_lo[:, t, :])
                    nc.gpsimd.tensor_sub(st_mid[:, t, :], tmid, st_hi[:, t, :])
                else:
                    tmid = tmpbp.tile([128, 512], f32, tag="tmidv",
                                      name=f"tmidv_{b}_{c}_{t}")
                    mid_eng.tensor_sub(tmid, x_sb[:, t, :], st_lo[:, t, :])
                    mid_eng.tensor_sub(st_mid[:, t, :], tmid, st_hi[:, t, :])

            def out_dmas(b, c, st_lo, st_hi, st_mid, split_mid=False):
                for st_sb, out_d in ((st_lo, low_d), (st_hi, high_d)):
                    oo = out_d[b, c].rearrange("(t p) w -> p t w", p=128)
                    nc.sync.dma_start(out=oo[:, 0:2, :], in_=st_sb[:, 0:2, :])
                    nc.sync.dma_start(out=oo[:, 2:4, :], in_=st_sb[:, 2:4, :])
                if split_mid:
                    mo = mid_d[b, c].rearrange("(t p) w -> p t w", p=128)
                    for t in range(4):
                        nc.sync.dma_start(out=mo[:, t:t + 1, :],
                                          in_=st_mid[:, t:t + 1, :])
                else:
                    mo = mid_d[b, c].rearrange("(t p) w -> p t w", p=128)
                    nc.sync.dma_start(out=mo[:, 0:2, :], in_=st_mid[:, 0:2, :])
                    nc.sync.dma_start(out=mo[:, 2:4, :], in_=st_mid[:, 2:4, :])

            def stage_tiles(b, c):
                st_lo = stagep.tile([128, 4, 512], f32, tag="stlo", name=f"stlo_{b}_{c}")
                st_hi = stagep.tile([128, 4, 512], f32, tag="sthi", name=f"sthi_{b}_{c}")
                st_mid = stagep.tile([128, 4, 512], f32, tag="stmid", name=f"stmid_{b}_{c}")
                return st_lo, st_hi, st_mid

            def finish_img(b, c):
                """apply masks + inverse transforms to staged coeffs."""
                st = state[b]
                st_lo, st_hi, st_mid = stage_tiles(b, c)
                for t in range(4):
                    finish_tile(b, c, t, st["coeff"][(c, t)],
                                st_lo, st_hi, st_mid, st["x"][c])
                out_dmas(b, c, st_lo, st_hi, st_mid)

            def chain_img(b, c, last=False):
                """full fused chain: DCT -> mask(from psum) -> inverse."""
                st = state[b]
                st_lo, st_hi, st_mid = stage_tiles(b, c)
                for t in range(4):
                    ps2 = dct_tile(b, c, t, st["x"][c])
                    finish_tile(b, c, t, ps2, st_lo, st_hi, st_mid, st["x"][c])
                out_dmas(b, c, st_lo, st_hi, st_mid, split_mid=last)

            # software-pipelined program order across the 2 batches
            load_imgs(0)
            load_imgs(1)
            dct_img(0, 0)
            pool_img(0, 0)
            dct_img(0, 1)
            pool_img(0, 1)
            pool_img(0, 2)
            mlp(0)
            finish_img(0, 0)
            pool_img(1, 0)
            finish_img(0, 1)
            pool_img(1, 1)
            chain_img(0, 2)
            pool_img(1, 2)
            mlp(1)
            chain_img(1, 0)
            chain_img(1, 1)
            chain_img(1, 2, last=True)

    nc.compile()
    _CACHE["nc"] = nc
    return nc


def kernel(**inputs):
    from concourse.bass_utils import run_bass_kernel_spmd

    x = np.ascontiguousarray(np.asarray(inputs["x"], dtype=np.float32).astype(np.float16))
    w1 = np.asarray(inputs["w1"], dtype=np.float32)
    b1 = np.asarray(inputs["b1"], dtype=np.float32)
    w2 = np.asarray(inputs["w2"], dtype=np.float32)
    b2 = np.asarray(inputs["b2"], dtype=np.float32)

    cst = _consts()
    nc = _build()

    # W1P[(c*8+jj), i, m] = w1[c*64+i*8+jj, m] / 4096
    w1s = (w1 / 4096.0).astype(np.float32).reshape(3, 8, 8, 32)  # [c, i, jj, m]
    W1P = np.ascontiguousarray(
        w1s.transpose(0, 2, 1, 3).reshape(24, 256))  # [(c jj), (i m)]
    C16 = np.zeros((128, 288), np.float16)
    C16[:, 0:128] = cst["BDT"].astype(np.float16)
    C16[:, 128:256] = cst["BDD"].astype(np.float16)
    C16[:, 256:288] = cst["ONESP"].astype(np.float16)
    C32 = np.zeros((128, 1088), np.float32)
    C32[:, 0:512] = cst["zzT"]
    C32[0:24, 512:768] = W1P
    C32[0:8, 768:776] = cst["ID8"]
    C32[0:2, 784:786] = cst["ID2"]
    C32[0:1, 800:928] = cst["ONES1"]
    C32[0:32, 928:930] = w2
    C32[0:32, 944:945] = b1.reshape(32, 1)
    C32[0:2, 960:961] = b2.reshape(2, 1)
    C32[0:2, 976:977] = cst["STHR"]
    C32[0:2, 992:993] = cst["BTHR"]
    C32[0:2, 1008:1009] = cst["SOUT"]
    C32[0:2, 1024:1025] = cst["BOUT"]
    base = {"C16": C16, "C32": C32}
    in_maps = []
    for i in range(NCORES):
        m = dict(base)
        m["x"] = np.ascontiguousarray(x[i * BPC:(i + 1) * BPC])
        in_maps.append(m)

    res = run_bass_kernel_spmd(nc, in_maps, core_ids=list(range(NCORES)))
    low = np.concatenate([res.results[i]["low"] for i in range(NCORES)], axis=0)
    mid = np.concatenate([res.results[i]["mid"] for i in range(NCORES)], axis=0)
    high = np.concatenate([res.results[i]["high"] for i in range(NCORES)], axis=0)
    low_t = np.concatenate([res.results[i]["low_t"] for i in range(NCORES)], axis=0)
    high_t = np.concatenate([res.results[i]["high_t"] for i in range(NCORES)], axis=0)
    return low, mid, high, (low_t, high_t)


# revision 37
# speedup vs baseline: 1.0492x; 1.0166x over previous
"""AdaptiveFrequencyDecomposition Trainium2 kernel (8 NeuronCores, pure data parallel).

Self-contained: hardcodes shapes B,C,H,W = 16,3,512,512, shards batch over 8 cores
(2 batches/core). Per 128x512 image tile the whole DCT->mask->IDCT chain runs as
4 matmul stages; transposes are fused into matmuls by using the *data* as the
stationary (lhsT) operand:
  stage1: psum1[:,128j:] = x_chunk_j^T @ blockdiag(D^T)   (col-DCT + transpose)
  stage2: psum2 = blockdiag(D^T)^T @ sb1                  (row-DCT) -> coeffs C^T layout
  stage3: psum3[:,128j:] = (mask*C^T)_chunk_j^T @ blockdiag(D)  (row-IDCT + transpose)
  stage4: psum4 = blockdiag(D)^T @ sb3                    (col-IDCT) -> natural layout
mid band is exact by linearity: mid = x - low - high (the reference's clip never
activates for these threshold ranges: max(low_mask+high_mask) < 1).
"""
import math
import os
import sys

for _p in ("/opt/trn_rl_repo",):
    if _p not in sys.path and os.path.isdir(_p):
        sys.path.append(_p)

import numpy as np

B, C, H, W = 16, 3, 512, 512
NCORES = 8
BPC = B // NCORES  # batches per core = 2


def _make_dct(n=8):
    d = np.zeros((n, n), dtype=np.float32)
    for k in range(n):
        for i in range(n):
            if k == 0:
                d[k, i] = 1.0 / math.sqrt(n)
            else:
                d[k, i] = math.sqrt(2.0 / n) * math.cos(math.pi * k * (2 * i + 1) / (2 * n))
    return d


def _make_zigzag(n=8):
    z = np.zeros((n, n), dtype=np.float32)
    i, j = 0, 0
    for idx in range(n * n):
        z[i, j] = idx
        if (i + j) % 2 == 0:
            if j == n - 1:
                i += 1
            elif i == 0:
                j += 1
            else:
                i -= 1
                j += 1
        elif i == n - 1:
            j += 1
        elif j == 0:
            i += 1
        else:
            i += 1
            j -= 1
    return z / (n * n - 1)


def _consts():
    D = _make_dct()
    ZZ = _make_zigzag()
    BDT = np.zeros((128, 128), np.float32)
    BDD = np.zeros((128, 128), np.float32)
    for g in range(16):
        BDT[g * 8:(g + 1) * 8, g * 8:(g + 1) * 8] = D.T
        BDD[g * 8:(g + 1) * 8, g * 8:(g + 1) * 8] = D
    p_idx = np.arange(128)
    f_idx = np.arange(512)
    zzT = ZZ[np.ix_(f_idx % 8, p_idx % 8)].T.astype(np.float32).copy()  # [128,512]
    ONESP = np.zeros((128, 32), np.float32)  # 4 blocks of [128, 8]
    for t in range(4):
        ONESP[:64, t * 8 + 2 * t] = 1.0
        ONESP[64:, t * 8 + 2 * t + 1] = 1.0
    ID8 = np.eye(8, dtype=np.float32)
    ID2 = np.eye(2, dtype=np.float32)
    ONES1 = np.ones((1, 128), np.float32)
    STHR = np.array([[12.5], [-12.5]], np.float32)
    BTHR = np.array([[7.5], [-30.0]], np.float32)
    SOUT = np.array([[0.25], [0.25]], np.float32)
    BOUT = np.array([[0.15], [0.6]], np.float32)
    return dict(BDT=BDT, BDD=BDD, zzT=zzT, ONESP=ONESP, ID8=ID8, ID2=ID2,
                ONES1=ONES1, STHR=STHR, BTHR=BTHR, SOUT=SOUT, BOUT=BOUT)


_CACHE = {}


def _build():
    """Build + compile the Bacc graph (once)."""
    if "nc" in _CACHE:
        return _CACHE["nc"]
    import concourse.bass as bass
    import concourse.mybir as mybir
    import concourse.tile as tile
    from concourse import bacc

    f32 = mybir.dt.float32
    f16 = mybir.dt.float16
    nc = bacc.Bacc("TRN2", target_bir_lowering=False, debug=False, num_devices=NCORES)

    # --- DRAM parameters ---
    x_d = nc.dram_tensor("x", [BPC, C, H, W], f16, kind="ExternalInput").ap()
    c16_d = nc.dram_tensor("C16", [128, 288], f16, kind="ExternalInput").ap()
    c32_d = nc.dram_tensor("C32", [128, 1088], f32, kind="ExternalInput").ap()

    low_d = nc.dram_tensor("low", [BPC, C, H, W], f32, kind="ExternalOutput").ap()
    mid_d = nc.dram_tensor("mid", [BPC, C, H, W], f32, kind="ExternalOutput").ap()
    high_d = nc.dram_tensor("high", [BPC, C, H, W], f32, kind="ExternalOutput").ap()
    lowt_d = nc.dram_tensor("low_t", [BPC, 1], f32, kind="ExternalOutput").ap()
    hight_d = nc.dram_tensor("high_t", [BPC, 1], f32, kind="ExternalOutput").ap()
    warm_d = nc.dram_tensor("warm", [128, 1], f32, kind="ExternalOutput").ap()


    Sig = mybir.ActivationFunctionType.Sigmoid
    Relu = mybir.ActivationFunctionType.Relu
    Ident = mybir.ActivationFunctionType.Identity
    AX = mybir.AxisListType.X

    with tile.TileContext(nc) as tc:
        with (
            tc.tile_pool(name="consts", bufs=1) as consts,
            tc.tile_pool(name="xin", bufs=7) as xin,
            tc.tile_pool(name="stage", bufs=3) as stagep,
            tc.tile_pool(name="coeff", bufs=4) as coeffp,
            tc.tile_pool(name="tmp", bufs=6) as tmpp,
            tc.tile_pool(name="tmpb", bufs=6) as tmpbp,
            tc.tile_pool(name="small", bufs=2) as smallp,
            tc.tile_pool(name="masks", bufs=2) as maskp,
            tc.tile_pool(name="ps1", bufs=1, space="PSUM") as ps1p,
            tc.tile_pool(name="ps2", bufs=2, space="PSUM") as ps2p,
            tc.tile_pool(name="ps3", bufs=2, space="PSUM") as ps3p,
            tc.tile_pool(name="ps4", bufs=2, space="PSUM") as ps4p,
            tc.tile_pool(name="psm", bufs=1, space="PSUM") as psmp,
        ):
            # --- PE warmup: dense junk matmul burst so HAM is at 2.4 GHz
            # when the real chains start (sunk to a dummy output vs DCE) ---
            warm_src = smallp.tile([128, 128], f32, tag="wsrc", name="warm_src")
            nc.vector.memset(warm_src, 1.0)
            warm_ps = ps4p.tile([128, 128], f32, tag="ps4", name="warm_ps")
            for i in range(10):
                nc.tensor.matmul(warm_ps, warm_src, warm_src,
                                 start=(i == 0), stop=(i == 9))
            warm_sb = smallp.tile([128, 1], f32, tag="wout", name="warm_sb")
            nc.vector.tensor_copy(out=warm_sb, in_=warm_ps[:, 0:1])
            nc.scalar.dma_start(out=warm_d, in_=warm_sb)

            # --- constants: two packed blob DMAs, sliced into views ---
            c16_sb = consts.tile([128, 288], f16, tag="c16", name="c16")
            nc.sync.dma_start(out=c16_sb, in_=c16_d)
            c32_sb = consts.tile([128, 1088], f32, tag="c32", name="c32")
            nc.sync.dma_start(out=c32_sb, in_=c32_d)
            bdt_sb = c16_sb[:, 0:128]
            bdd_sb = c16_sb[:, 128:256]
            onesp_sb = c16_sb[:, 256:288]
            zzt_sb = c32_sb[:, 0:512]
            w1p_sb = c32_sb[0:24, 512:768]
            id8_sb = c32_sb[0:8, 768:776]
            id2_sb = c32_sb[0:2, 784:786]
            ones1_sb = c32_sb[0:1, 800:928]
            w2_sb = c32_sb[0:32, 928:930]
            b1_sb = c32_sb[0:32, 944:945]
            b2_sb = c32_sb[0:2, 960:961]
            sthr_sb = c32_sb[0:2, 976:977]
            bthr_sb = c32_sb[0:2, 992:993]
            sout_sb = c32_sb[0:2, 1008:1009]
            bout_sb = c32_sb[0:2, 1024:1025]

            state = {}  # per-batch: x tiles, masks

            def load_imgs(b):
                st = state.setdefault(b, {"x": {}})
                for c in range(C):
                    x_sb = xin.tile([128, 4, 512], f16, tag="x", name=f"x_{b}_{c}")
                    nc.scalar.dma_start(
                        out=x_sb,
                        in_=x_d[b, c].rearrange("(t p) w -> p t w", p=128),
                    )
                    st["x"][c] = x_sb

            def pool_img(b, c):
                st = state[b]
                x_sb = st["x"][c]
                pool_ps = psmp.tile([8, 512], f32, tag="psm", name=f"poolps_{b}_{c}")
                for t in range(4):
                    nc.tensor.matmul(
                        pool_ps,
                        onesp_sb[:, t * 8:(t + 1) * 8],
                        x_sb[:, t, :],
                        start=(t == 0),
                        stop=(t == 3),
                    )
                pooled_all = st.setdefault(
                    "pooled",
                    smallp.tile([8, 24], f32, tag="pooled_all", name=f"pooled_{b}"))
                nc.vector.reduce_sum(
                    out=pooled_all[:, c * 8:(c + 1) * 8],
                    in_=pool_ps.rearrange("p (jj w) -> p jj w", w=64),
                    axis=AX,
                )

            def mlp(b):
                st = state[b]
                pooled_all = st["pooled"]
                pooledT_ps = psmp.tile([24, 8], f32, tag="psm", name=f"pooledT_ps_{b}")
                nc.tensor.matmul(pooledT_ps, pooled_all, id8_sb)
                pooledT_sb = smallp.tile([24, 8], f32, tag="pooledT", name=f"pooledT_{b}")
                nc.vector.tensor_copy(out=pooledT_sb, in_=pooledT_ps)
                # h[m] = sum_i sum_(c,jj) pooledT[(c jj), i] * W1P[(c jj), i, m]
                h_ps = psmp.tile([32, 1], f32, tag="psm", name=f"h_ps_{b}")
                for i in range(8):
                    nc.tensor.matmul(
                        h_ps, w1p_sb[:, i * 32:(i + 1) * 32], pooledT_sb[:, i:i + 1],
                        start=(i == 0), stop=(i == 7),
                    )
                h_sb = smallp.tile([32, 1], f32, tag="h", name=f"h_{b}")
                nc.scalar.activation(out=h_sb, in_=h_ps, func=Relu, bias=b1_sb, scale=1.0)
                t_ps = psmp.tile([2, 1], f32, tag="psm", name=f"t_ps_{b}")
                nc.tensor.matmul(t_ps, w2_sb, h_sb)
                tt_sb = smallp.tile([2, 1], f32, tag="tt", name=f"tt_{b}")
                nc.scalar.activation(out=tt_sb, in_=t_ps, func=Sig, bias=b2_sb, scale=1.0)
                # thr = tt*[12.5,-12.5] + [7.5,-30]  (= [50*low_t, -50*high_t])
                thr_sb = smallp.tile([2, 1], f32, tag="thr", name=f"thr_{b}")
                nc.scalar.activation(out=thr_sb, in_=tt_sb, func=Ident,
                                     bias=bthr_sb, scale=sthr_sb)
                # unscaled thresholds out
                tout_sb = smallp.tile([2, 1], f32, tag="tout", name=f"tout_{b}")
                nc.scalar.activation(out=tout_sb, in_=tt_sb, func=Ident,
                                     bias=bout_sb, scale=sout_sb)
                nc.sync.dma_start(out=lowt_d[b:b + 1, :], in_=tout_sb[0:1, 0:1])
                nc.sync.dma_start(out=hight_d[b:b + 1, :], in_=tout_sb[1:2, 0:1])
                # broadcast thresholds across partitions: transpose then ones-matmul
                thrT_ps = psmp.tile([1, 2], f32, tag="psm", name=f"thrT_ps_{b}")
                nc.tensor.matmul(thrT_ps, thr_sb, id2_sb)
                thrT_sb = smallp.tile([1, 2], f32, tag="thrT", name=f"thrT_{b}")
                nc.vector.tensor_copy(out=thrT_sb, in_=thrT_ps)
                bc_ps = psmp.tile([128, 2], f32, tag="psm", name=f"bc_ps_{b}")
                nc.tensor.matmul(bc_ps, ones1_sb, thrT_sb)
                bc_sb = smallp.tile([128, 2], f32, tag="bc", name=f"bc_{b}")
                nc.vector.tensor_copy(out=bc_sb, in_=bc_ps)
                mask_lo = maskp.tile([128, 512], f16, tag="mlo", name=f"mlo_{b}")
                nc.scalar.activation(out=mask_lo, in_=zzt_sb, func=Sig,
                                     bias=bc_sb[:, 0:1], scale=-50.0)
                mask_hi = maskp.tile([128, 512], f16, tag="mhi", name=f"mhi_{b}")
                nc.scalar.activation(out=mask_hi, in_=zzt_sb, func=Sig,
                                     bias=bc_sb[:, 1:2], scale=50.0)
                st["mask_lo"] = mask_lo
                st["mask_hi"] = mask_hi

            def dct_tile(b, c, t, x_sb):
                """stage1+stage2 for one tile; returns the coeff psum tile."""
                ps1 = ps1p.tile([128, 512], f32, tag="ps1", name=f"ps1_{b}_{c}_{t}")
                for j in range(4):
                    nc.tensor.matmul(
                        ps1[:, 128 * j:128 * (j + 1)],
                        x_sb[:, t, 128 * j:128 * (j + 1)],
                        bdt_sb,
                    )
                sb1 = tmpp.tile([128, 512], f16, tag="sb1", name=f"sb1_{b}_{c}_{t}")
                nc.any.tensor_copy(out=sb1, in_=ps1)
                ps2 = ps2p.tile([128, 512], f32, tag="ps2", name=f"ps2_{b}_{c}_{t}")
                nc.tensor.matmul(ps2, bdt_sb, sb1)
                return ps2

            def dct_img(b, c):
                """early DCT: stage coeffs to SBUF (before masks are known)."""
                st = state[b]
                cf = st.setdefault("coeff", {})
                for t in range(4):
                    ps2 = dct_tile(b, c, t, st["x"][c])
                    coeff_sb = coeffp.tile([128, 512], f16, tag="coeff",
                                           name=f"coeff_{b}_{c}_{t}")
                    nc.any.tensor_copy(out=coeff_sb, in_=ps2)
                    cf[(c, t)] = coeff_sb

            def inv_half(b, c, t, msk_sl, st_sb, mn):
                ps3 = ps3p.tile([128, 512], f32, tag="ps3",
                                name=f"ps3_{b}_{c}_{t}_{mn}")
                for j in range(4):
                    nc.tensor.matmul(
                        ps3[:, 128 * j:128 * (j + 1)],
                        msk_sl[:, 128 * j:128 * (j + 1)],
                        bdd_sb,
                    )
                sb3 = tmpp.tile([128, 512], f16, tag="sb3",
                                name=f"sb3_{b}_{c}_{t}_{mn}")
                nc.any.tensor_copy(out=sb3, in_=ps3)
                ps4 = ps4p.tile([128, 512], f32, tag="ps4",
                                name=f"ps4_{b}_{c}_{t}_{mn}")
                nc.tensor.matmul(ps4, bdd_sb, sb3)
                nc.any.tensor_copy(out=st_sb[:, t, :], in_=ps4)

            def finish_tile(b, c, t, coeff_src, st_lo, st_hi, st_mid, x_sb,
                            mid_eng=None):
                for mask_key, st_sb, mn in (("mask_lo", st_lo, "lo"),
                                            ("mask_hi", st_hi, "hi")):
                    msk = tmpbp.tile([128, 512], f16, tag="msk",
                                     name=f"msk_{b}_{c}_{t}_{mn}")
                    nc.vector.tensor_mul(msk, coeff_src, state[b][mask_key])
                    inv_half(b, c, t, msk, st_sb, mn)
                if mid_eng is None:
                    tmid = tmpbp.tile([128, 512], f16, tag="tmid",
                                      name=f"tmid_{b}_{c}_{t}")
                    nc.gpsimd.tensor_sub(tmid, x_sb[:, t, :], st_lo[:, t, :])
                    nc.gpsimd.tensor_sub(st_mid[:, t, :], tmid, st_hi[:, t, :])
                else:
                    tmid = tmpbp.tile([128, 512], f32, tag="tmidv",
                                      name=f"tmidv_{b}_{c}_{t}")
                    mid_eng.tensor_sub(tmid, x_sb[:, t, :], st_lo[:, t, :])
                    mid_eng.tensor_sub(st_mid[:, t, :], tmid, st_hi[:, t, :])

            def out_dmas(b, c, st_lo, st_hi, st_mid, split_mid=False):
                for st_sb, out_d in ((st_lo, low_d), (st_hi, high_d)):
                    oo = out_d[b, c].rearrange("(t p) w -> p t w", p=128)
                    nc.sync.dma_start(out=oo[:, 0:2, :], in_=st_sb[:, 0:2, :])
                    nc.sync.dma_start(out=oo[:, 2:4, :], in_=st_sb[:, 2:4, :])
                if split_mid:
                    mo = mid_d[b, c].rearrange("(t p) w -> p t w", p=128)
                    for t in range(4):
                        nc.sync.dma_start(out=mo[:, t:t + 1, :],
                                          in_=st_mid[:, t:t + 1, :])
                else:
                    mo = mid_d[b, c].rearrange("(t p) w -> p t w", p=128)
                    nc.sync.dma_start(out=mo[:, 0:2, :], in_=st_mid[:, 0:2, :])
                    nc.sync.dma_start(out=mo[:, 2:4, :], in_=st_mid[:, 2:4, :])

            def stage_tiles(b, c):
                st_lo = stagep.tile([128, 4, 512], f32, tag="stlo", name=f"stlo_{b}_{c}")
                st_hi = stagep.tile([128, 4, 512], f32, tag="sthi", name=f"sthi_{b}_{c}")
                st_mid = stagep.tile([128, 4, 512], f32, tag="stmid", name=f"stmid_{b}_{c}")
                return st_lo, st_hi, st_mid

            def finish_img(b, c):
                """apply masks + inverse transforms to staged coeffs."""
                st = state[b]
                st_lo, st_hi, st_mid = stage_tiles(b, c)
                for t in range(4):
                    finish_tile(b, c, t, st["coeff"][(c, t)],
                                st_lo, st_hi, st_mid, st["x"][c])
                out_dmas(b, c, st_lo, st_hi, st_mid)

            def chain_img(b, c, last=False):
                """full fused chain: DCT -> mask(from psum) -> inverse."""
                st = state[b]
                st_lo, st_hi, st_mid = stage_tiles(b, c)
                for t in range(4):
                    ps2 = dct_tile(b, c, t, st["x"][c])
                    finish_tile(b, c, t, ps2, st_lo, st_hi, st_mid, st["x"][c])
                out_dmas(b, c, st_lo, st_hi, st_mid, split_mid=last)

            # software-pipelined program order across the 2 batches
            load_imgs(0)
            load_imgs(1)
            # latency-critical threshold chains first: high scheduler priority
            # lets every MLP hop jump the engine queues
            pool_img(0, 0)
            pool_img(0, 1)
            pool_img(0, 2)
            mlp(0)
            pool_img(1, 0)
            pool_img(1, 1)
            pool_img(1, 2)
            mlp(1)
            dct_img(0, 0)
            finish_img(0, 0)
            chain_img(0, 1)
            chain_img(0, 2)
            chain_img(1, 0)
            chain_img(1, 1)
            chain_img(1, 2, last=True)

    nc.compile()
    _CACHE["nc"] = nc
    return nc


def kernel(**inputs):
    from concourse.bass_utils import run_bass_kernel_spmd

    x = np.ascontiguousarray(np.asarray(inputs["x"], dtype=np.float32).astype(np.float16))
    w1 = np.asarray(inputs["w1"], dtype=np.float32)
    b1 = np.asarray(inputs["b1"], dtype=np.float32)
    w2 = np.asarray(inputs["w2"], dtype=np.float32)
    b2 = np.asarray(inputs["b2"], dtype=np.float32)

    cst = _consts()
    nc = _build()

    # W1P[(c*8+jj), i, m] = w1[c*64+i*8+jj, m] / 4096
    w1s = (w1 / 4096.0).astype(np.float32).reshape(3, 8, 8, 32)  # [c, i, jj, m]
    W1P = np.ascontiguousarray(
        w1s.transpose(0, 2, 1, 3).reshape(24, 256))  # [(c jj), (i m)]
    C16 = np.zeros((128, 288), np.float16)
    C16[:, 0:128] = cst["BDT"].astype(np.float16)
    C16[:, 128:256] = cst["BDD"].astype(np.float16)
    C16[:, 256:288] = cst["ONESP"].astype(np.float16)
    C32 = np.zeros((128, 1088), np.float32)
    C32[:, 0:512] = cst["zzT"]
    C32[0:24, 512:768] = W1P
    C32[0:8, 768:776] = cst["ID8"]
    C32[0:2, 784:786] = cst["ID2"]
    C32[0:1, 800:928] = cst["ONES1"]
    C32[0:32, 928:930] = w2
    C32[0:32, 944:945] = b1.reshape(32, 1)
    C32[0:2, 960:961] = b2.reshape(2, 1)
    C32[0:2, 976:977] = cst["STHR"]
    C32[0:2, 992:993] = cst["BTHR"]
    C32[0:2, 1008:1009] = cst["SOUT"]
    C32[0:2, 1024:1025] = cst["BOUT"]
    base = {"C16": C16, "C32": C32}
    in_maps = []
    for i in range(NCORES):
        m = dict(base)
        m["x"] = np.ascontiguousarray(x[i * BPC:(i + 1) * BPC])
        in_maps.append(m)

    res = run_bass_kernel_spmd(nc, in_maps, core_ids=list(range(NCORES)))
    low = np.concatenate([res.results[i]["low"] for i in range(NCORES)], axis=0)
    mid = np.concatenate([res.results[i]["mid"] for i in range(NCORES)], axis=0)
    high = np.concatenate([res.results[i]["high"] for i in range(NCORES)], axis=0)
    low_t = np.concatenate([res.results[i]["low_t"] for i in range(NCORES)], axis=0)
    high_t = np.concatenate([res.results[i]["high_t"] for i in range(NCORES)], axis=0)
    return low, mid, high, (low_t, high_t)
